# revision 11
# baseline (speedup 1.0000x reference)
"""DeepSeek-V3 MoE gate (nn_MoEGate) Trainium2 Bass kernel — v4.

Math (per token): logits = x @ w; s = sigmoid(logits) + bias;
hierarchical top-k: per-group top-2 sums -> top-4 groups -> mask ->
top-8 experts; weights = normalized masked scores * 2.5.

Numerics identical to v2 (2.0 byte-passes). v4 schedule:
  - 384-token chunks (+128 tail), x planes chunk-contiguous: each pass
    is one full-width matmul stream (fp16 moving 384, DR moving [2,384])
    -> fewest instructions, 2 DMA transfers per chunk.
  - PSUM group rule (one open accumulation group per bank): fp16 kt0
    opens the bank full-width (start=True), DR-B kp27 closes it
    (stop=True); DR-A sits in between, so its fp8 plane is only needed
    mid-chunk and can be single-buffered.
  - pm tiles are full banks [128,512] (sliced to cs) so the e0/e1
    accumulation groups never share a bank.
  - Small 128-token tail chunk -> small exposed top-k tail.

Matmul scheme (logits*2^19 = pass1 + DR-A + DR-B, w' = 0.5*w):
  pass1: x_hi16 (fp16(x*2^8), moving) . w_hi16 (fp16(w*2^11), stationary)
  DR-A : e4m3((x*2^8 - x_hi16)*2^8) . e4m3(w*2^3), fp8 DoubleRow pairs
  DR-B : e5m2 view of x_hi16's high bytes . e5m2(w*2^11 - w_hi16)
sigmoid via tanh identity (s2 = tanh + 1 + 2*bias); hardware Max8 /
MaxIndex top-k on DVE. Token-parallel across 8 cores (2048 tokens each).
"""
import numpy as np

import concourse.bass as bass
import concourse.mybir as mybir
import concourse.tile as tile
from concourse.bass_utils import run_bass_kernel_spmd
from concourse.dt import dt as cdt
from concourse.masks import make_identity

F32 = mybir.dt.float32
F16 = mybir.dt.float16
F8E4 = mybir.dt.float8e4
F8E5 = mybir.dt.float8e5
U32 = mybir.dt.uint32

NP_E4 = cdt.np(F8E4)
NP_E5 = cdt.np(F8E5)

N_CORES = 8
BSZ, SEQ, H = 4, 4096, 7168
N_TOK = BSZ * SEQ                  # 16384
TPC = N_TOK // N_CORES             # 2048 tokens per core
E = 256                            # experts
G, EPG = 8, 32                     # groups, experts/group
CHUNKS = (384, 384, 384, 384, 384, 128)  # token chunks per core (sum=TPC)
KT = H // 128                      # 56 k-tiles
KP = KT // 2                       # 28 k-tile pairs

SCALE_X = 2.0 ** 8
SCALE_W = 2.0 ** 11                # w * 0.5 * 2^12
ACT_SCALE = 2.0 ** -20             # undo 2^19, x.w*0.5
ROUTED_SCALING = 2.5
DR = mybir.MatmulPerfMode.DoubleRow


def _split_caps(nc):
    """Split >1-wait sync_info into standalone EventSemaphore insts.

    This walrus build accepts at most one sem wait per engine
    instruction (EventSemaphore holds two)."""
    n = 0
    for fn in nc.m.functions:
        for bb in fn.blocks:
            insts = bb.instructions
            new = []
            changed = False
            for inst in insts:
                si = inst.sync_info
                waits = list(si.on_wait) if si is not None and si.on_wait else []
                if len(waits) > 1 and str(inst.opcode) != "EventSemaphore":
                    excess, keep = waits[:-1], waits[-1:]
                    for i in range(0, len(excess), 2):
                        ev = mybir.InstEventSemaphore(
                            name=f"EVW-{inst.name}-{i}", engine=inst.engine
                        )
                        ev.sync_info = mybir.SyncInfo(
                            on_wait=excess[i:i + 2], on_update=[]
                        )
                        new.append(ev)
                        n += 1
                    inst.sync_info = mybir.SyncInfo(
                        on_wait=keep,
                        on_update=list(si.on_update) if si.on_update else [],
                    )
                    changed = True
                new.append(inst)
            if changed:
                insts[:] = new
    return n


def build_nc(repeat=1, mode="full", chunks=CHUNKS):
    starts = np.cumsum([0] + list(chunks))[:-1]
    nc = bass.Bass("TRN2", target_bir_lowering=False, debug=False)

    # x planes, chunk-contiguous; h = kt*128 + p
    XHs = [
        nc.dram_tensor(f"XH{c}", [128, KT, cs], F16, kind="ExternalInput").ap()
        for c, cs in enumerate(chunks)
    ]
    XLs = [
        nc.dram_tensor(
            f"XL{c}", [128, KP, 2, cs], F8E4, kind="ExternalInput").ap()
        for c, cs in enumerate(chunks)
    ]
    WH = nc.dram_tensor("WH", [128, KT, E], F16, kind="ExternalInput").ap()
    W8A = nc.dram_tensor("W8A", [128, KP, 2, E], F8E4, kind="ExternalInput").ap()
    W8B = nc.dram_tensor("W8B", [128, KP, 2, E], F8E5, kind="ExternalInput").ap()
    B2 = nc.dram_tensor("B2", [128, E], F32, kind="ExternalInput").ap()

    OIDX = nc.dram_tensor("OIDX", [TPC, 8], U32, kind="ExternalOutput").ap()
    OWTS = nc.dram_tensor("OWTS", [TPC, 8], F32, kind="ExternalOutput").ap()

    with tile.TileContext(nc) as tc:
        with (
            tc.tile_pool(name="const", bufs=1) as cpool,
            tc.tile_pool(name="xh", bufs=2) as xhpool,
            tc.tile_pool(name="xl", bufs=1) as xlpool,
            tc.tile_pool(name="pm", bufs=2, space="PSUM") as pmpool,
            tc.tile_pool(name="pt", bufs=3, space="PSUM") as ptpool,
            tc.tile_pool(name="ts", bufs=2) as tpool,
            tc.tile_pool(name="sc", bufs=2) as spool,
            tc.tile_pool(name="sm", bufs=3) as smpool,
            tc.tile_pool(name="out", bufs=1) as opool,
        ):
            # weights DMA'd once, outside the repeat loop
            wh = cpool.tile([128, KT, E], F16, name="wh")
            nc.sync.dma_start(wh[:], WH)
            w8a = cpool.tile([128, KP, 2, E], F8E4, name="w8a")
            nc.sync.dma_start(w8a[:], W8A)
            w8b = cpool.tile([128, KP, 2, E], F8E5, name="w8b")
            nc.sync.dma_start(w8b[:], W8B)
            bias2 = cpool.tile([128, E], F32, name="bias2")
            nc.sync.dma_start(bias2[:], B2)
            ident = cpool.tile([128, 128], F32, name="ident")
            make_identity(nc, ident[:])

            oidx = opool.tile([128, TPC // 128, 8], U32, name="oidx")
            owts = opool.tile([128, TPC // 128, 8], F32, name="owts")

            xh_tiles = {}   # c -> tile [128, KT, cs]
            xl_tiles = {}   # c -> tile [128, KP, 2, cs]
            pms = {}        # c -> [pm0, pm1]

            def dma_xh(c):
                cs = chunks[c]
                t = xhpool.tile([128, KT, cs], F16, tag="xh", name=f"xh_{c}")
                xh_tiles[c] = t
                nc.sync.dma_start(t[:], XHs[c])

            def dma_xl(c):
                cs = chunks[c]
                t = xlpool.tile([128, KP, 2, cs], F8E4, tag="xl",
                                name=f"xl_{c}")
                xl_tiles[c] = t
                nc.sync.dma_start(t[:], XLs[c])

            def mm_phase(c):
                cs = chunks[c]
                pm = [
                    pmpool.tile([128, 512], F32, tag=f"pm{e}",
                                name=f"pm{e}_{c}")
                    for e in (0, 1)
                ]
                pms[c] = pm
                xh = xh_tiles.pop(c)
                xt = xl_tiles.pop(c)
                # fp16 pass; kt0 opens the full-width accumulation group
                for kt in range(KT):
                    for e in (0, 1):
                        nc.tensor.matmul(
                            pm[e][:, 0:cs],
                            wh[:, kt, e * 128:(e + 1) * 128],
                            xh[:, kt, :],
                            start=(kt == 0), stop=False,
                        )
                # DR-A, chunk-wide moving [2, cs]
                for kp in range(KP):
                    for e in (0, 1):
                        nc.tensor.matmul(
                            pm[e][:, 0:cs],
                            w8a[:, kp, :, e * 128:(e + 1) * 128],
                            xt[:, kp, :, :],
                            start=False, stop=False,
                            perf_mode=DR,
                        )
                # DR-B: e5m2 high-byte view of the fp16 plane; kp27 closes
                # the accumulation group
                xv = xh[:].bitcast(F8E5).rearrange("p k (n b) -> p k b n", b=2)
                for kp in range(KP):
                    for e in (0, 1):
                        nc.tensor.matmul(
                            pm[e][:, 0:cs],
                            w8b[:, kp, :, e * 128:(e + 1) * 128],
                            xv[:, 2 * kp:2 * kp + 2, 1, :],
                            start=False, stop=(kp == KP - 1),
                            perf_mode=DR,
                        )

            def post_phase(c):
                cs = chunks[c]
                if mode in ("mm_only",):
                    pms.pop(c)
                    return
                pm = pms.pop(c)
                # tanh(x.w/2) -> SBUF, [e, tok] layout
                ts = []
                for e in (0, 1):
                    t = tpool.tile([128, 512], F32, tag="ts", name=f"t{e}_{c}")
                    nc.scalar.activation(
                        t[:, 0:cs], pm[e][:, 0:cs],
                        mybir.ActivationFunctionType.Tanh,
                        scale=ACT_SCALE,
                    )
                    ts.append(t)
                # phase A: transpose to [tok, e] and add bias -- kept ahead
                # of the long DVE chains so PE never stalls on DVE slots
                s2s = []
                for j in range(cs // 128):
                    pt = ptpool.tile([128, E], F32, tag="pt",
                                     name=f"pt_{c}_{j}")
                    for e in (0, 1):
                        nc.tensor.matmul(
                            pt[:, e * 128:(e + 1) * 128],
                            ts[e][:, j * 128:(j + 1) * 128],
                            ident[:],
                            is_transpose=True,
                            start=(e == 0),
                            stop=(e == 1),
                        )
                    # s2 = 2*sigmoid + 2*bias = tanh + (1 + 2*bias)
                    s2 = spool.tile([128, E], F32, tag="s2",
                                    name=f"s2_{c}_{j}")
                    nc.vector.tensor_tensor(
                        s2[:], pt[:], bias2[:], op=mybir.AluOpType.add
                    )
                    s2s.append(s2)
                if mode == "half_post":
                    return
                # phase B: per-token hierarchical top-k (pure DVE; overlaps
                # the next chunk's matmuls)
                for j in range(cs // 128):
                    tok0 = int(starts[c]) // 128 + j
                    s2 = s2s[j]
                    # group scores: top-2 sum within each group of 32
                    g8 = smpool.tile([128, G, 8], F32, tag="g8",
                                     name=f"g8_{c}_{j}")
                    for g in range(G):
                        nc.vector.max(
                            out=g8[:, g, :], in_=s2[:, g * EPG:(g + 1) * EPG]
                        )
                    gs = smpool.tile([128, G], F32, tag="gs",
                                     name=f"gs_{c}_{j}")
                    nc.vector.reduce_sum(
                        gs[:], g8[:, :, 0:2], axis=mybir.AxisListType.X
                    )
                    gss = smpool.tile([128, G], F32, tag="gss",
                                      name=f"gss_{c}_{j}")
                    nc.vector.max(out=gss[:], in_=gs[:])
                    gmask = smpool.tile([128, G], F32, tag="gmask",
                                        name=f"gm_{c}_{j}")
                    nc.vector.tensor_scalar(
                        gmask[:], gs[:], gss[:, 3:4], None,
                        op0=mybir.AluOpType.is_ge,
                    )
                    s2m = spool.tile([128, E], F32, tag="s2m",
                                     name=f"s2m_{c}_{j}")
                    nc.vector.tensor_tensor(
                        s2m[:].rearrange("p (g e) -> p g e", g=G),
                        s2[:].rearrange("p (g e) -> p g e", g=G),
                        gmask[:].to_broadcast([128, G, EPG]),
                        op=mybir.AluOpType.mult,
                    )
                    # top-8 experts
                    mx = smpool.tile([128, 8], F32, tag="mx",
                                     name=f"mx_{c}_{j}")
                    nc.vector.max(out=mx[:], in_=s2m[:])
                    nc.vector.max_index(
                        out=oidx[:, tok0, :], in_max=mx[:], in_values=s2m[:]
                    )
                    # normalize: w = mx / sum(mx) * 2.5
                    sm = smpool.tile([128, 1], F32, tag="sm",
                                     name=f"sm_{c}_{j}")
                    nc.vector.reduce_sum(sm[:], mx[:], axis=mybir.AxisListType.X)
                    rc = smpool.tile([128, 1], F32, tag="rc",
                                     name=f"rc_{c}_{j}")
                    nc.vector.reciprocal(rc[:], sm[:])
                    nc.vector.tensor_scalar(
                        owts[:, tok0, :], mx[:], rc[:, 0:1], ROUTED_SCALING,
                        op0=mybir.AluOpType.mult, op1=mybir.AluOpType.mult,
                    )

            def trace_all():
                # xh double-buffered (prefetch c+1 during c); xl single-
                # buffered: chunk c+1's xl DMA is issued after chunk c's
                # DR-A (xl's last reader) and only needs to land by c+1's
                # DR-A, ~60% through its PE stream. post(c-1) is emitted
                # after mm(c) so PE transposes/DVE top-k have slack.
                dma_xh(0)
                dma_xl(0)
                dma_xh(1)
                for c in range(len(chunks)):
                    mm_phase(c)
                    if c + 1 < len(chunks):
                        dma_xl(c + 1)
                    if c + 2 < len(chunks):
                        dma_xh(c + 2)
                    if c > 0:
                        post_phase(c - 1)
                post_phase(len(chunks) - 1)

            if mode != "full":
                nc.gpsimd.memset(oidx[:], 0)
                nc.gpsimd.memset(owts[:], 0.0)
            if repeat == 1:
                trace_all()
            else:
                with tc.For_i(0, repeat, 1):
                    trace_all()

            nc.sync.dma_start(
                OIDX.rearrange("(t p) k -> p t k", p=128), oidx[:]
            )
            nc.sync.dma_start(
                OWTS.rearrange("(t p) k -> p t k", p=128), owts[:]
            )

    _split_caps(nc)
    return nc


def prep_inputs(hidden_states, weight, bias, chunks=CHUNKS):
    """Host-side: scale, fp16+fp8 plane split, transpose, per-core layout."""
    x = np.ascontiguousarray(hidden_states, dtype=np.float32).reshape(N_TOK, H)

    wf = weight.astype(np.float32)
    wh_f32 = (wf * SCALE_W).astype(np.float16).astype(np.float32)   # w*2^11 rounded
    # WH [H, E] fp16 -> [128, KT, E]
    WHm = np.ascontiguousarray(
        wh_f32.astype(np.float16).reshape(KT, 128, E).transpose(1, 0, 2))
    # W8A = e4m3(w*2^3) -> [128, KP, 2, E]
    W8Am = np.ascontiguousarray(
        (wf * 8.0).astype(NP_E4).reshape(KP, 2, 128, E).transpose(2, 0, 1, 3))
    # W8B = e5m2(w*2^11 - wh) -> [128, KP, 2, E]
    wlo = wf * SCALE_W - wh_f32
    W8Bm = np.ascontiguousarray(
        wlo.astype(NP_E5).reshape(KP, 2, 128, E).transpose(2, 0, 1, 3))

    b2 = (1.0 + 2.0 * bias.astype(np.float32))[None, :]
    b2 = np.ascontiguousarray(np.broadcast_to(b2, (128, E)))

    starts = np.cumsum([0] + list(chunks))[:-1]
    in_maps = []
    for c in range(N_CORES):
        xc = x[c * TPC:(c + 1) * TPC] * SCALE_X          # [TPC, H] f32
        xh = xc.astype(np.float16)
        r8 = ((xc - xh.astype(np.float32)) * 256.0).astype(NP_E4)
        xht = xh.T.reshape(KT, 128, TPC)                 # [KT, 128, TPC]
        r8t = r8.T.reshape(KP, 2, 128, TPC)              # [KP, 2, 128, TPC]
        im = dict(WH=WHm, W8A=W8Am, W8B=W8Bm, B2=b2)
        for ci, cs in enumerate(chunks):
            t0 = int(starts[ci])
            im[f"XH{ci}"] = np.ascontiguousarray(
                xht[:, :, t0:t0 + cs].transpose(1, 0, 2))
            im[f"XL{ci}"] = np.ascontiguousarray(
                r8t[:, :, :, t0:t0 + cs].transpose(2, 0, 1, 3))
        in_maps.append(im)
    return in_maps


_NC_CACHE = {}


def kernel(hidden_states, weight, bias):
    key = "main"
    if key not in _NC_CACHE:
        _NC_CACHE[key] = build_nc()
    nc = _NC_CACHE[key]
    in_maps = prep_inputs(hidden_states, weight, bias)
    res = run_bass_kernel_spmd(nc, in_maps, core_ids=list(range(N_CORES)))
    idx = np.concatenate(
        [r["OIDX"].astype(np.int32) for r in res.results], axis=0
    ).reshape(N_TOK, 8)
    wts = np.concatenate([r["OWTS"] for r in res.results], axis=0).reshape(N_TOK, 8)
    return idx, wts


# revision 12
# speedup vs baseline: 1.0730x; 1.0730x over previous
"""DeepSeek-V3 MoE gate (nn_MoEGate) Trainium2 Bass kernel — v4.

Math (per token): logits = x @ w; s = sigmoid(logits) + bias;
hierarchical top-k: per-group top-2 sums -> top-4 groups -> mask ->
top-8 experts; weights = normalized masked scores * 2.5.

Numerics identical to v2 (2.0 byte-passes). v4 schedule:
  - 384-token chunks (+128 tail), x planes chunk-contiguous: each pass
    is one full-width matmul stream (fp16 moving 384, DR moving [2,384])
    -> fewest instructions, 2 DMA transfers per chunk.
  - PSUM group rule (one open accumulation group per bank): fp16 kt0
    opens the bank full-width (start=True), DR-B kp27 closes it
    (stop=True); DR-A sits in between, so its fp8 plane is only needed
    mid-chunk and can be single-buffered.
  - pm tiles are full banks [128,512] (sliced to cs) so the e0/e1
    accumulation groups never share a bank.
  - Small 128-token tail chunk -> small exposed top-k tail.

Matmul scheme (logits*2^19 = pass1 + DR-A + DR-B, w' = 0.5*w):
  pass1: x_hi16 (fp16(x*2^8), moving) . w_hi16 (fp16(w*2^11), stationary)
  DR-A : e4m3((x*2^8 - x_hi16)*2^8) . e4m3(w*2^3), fp8 DoubleRow pairs
  DR-B : e5m2 view of x_hi16's high bytes . e5m2(w*2^11 - w_hi16)
sigmoid via tanh identity (s2 = tanh + 1 + 2*bias); hardware Max8 /
MaxIndex top-k on DVE. Token-parallel across 8 cores (2048 tokens each).
"""
import numpy as np

import concourse.bass as bass
import concourse.mybir as mybir
import concourse.tile as tile
from concourse.bass_utils import run_bass_kernel_spmd
from concourse.dt import dt as cdt
from concourse.masks import make_identity

F32 = mybir.dt.float32
F16 = mybir.dt.float16
F8E4 = mybir.dt.float8e4
F8E5 = mybir.dt.float8e5
U32 = mybir.dt.uint32

NP_E4 = cdt.np(F8E4)
NP_E5 = cdt.np(F8E5)

N_CORES = 8
BSZ, SEQ, H = 4, 4096, 7168
N_TOK = BSZ * SEQ                  # 16384
TPC = N_TOK // N_CORES             # 2048 tokens per core
E = 256                            # experts
G, EPG = 8, 32                     # groups, experts/group
CHUNKS = (384, 384, 384, 384, 384, 128)  # token chunks per core (sum=TPC)
KT = H // 128                      # 56 k-tiles
KP = KT // 2                       # 28 k-tile pairs

SCALE_X = 2.0 ** 8
SCALE_W = 2.0 ** 11                # w * 0.5 * 2^12
ACT_SCALE = 2.0 ** -20             # undo 2^19, x.w*0.5
ROUTED_SCALING = 2.5
DR = mybir.MatmulPerfMode.DoubleRow


def _split_caps(nc):
    """Split >1-wait sync_info into standalone EventSemaphore insts.

    This walrus build accepts at most one sem wait per engine
    instruction (EventSemaphore holds two)."""
    n = 0
    for fn in nc.m.functions:
        for bb in fn.blocks:
            insts = bb.instructions
            new = []
            changed = False
            for inst in insts:
                si = inst.sync_info
                waits = list(si.on_wait) if si is not None and si.on_wait else []
                if len(waits) > 1 and str(inst.opcode) != "EventSemaphore":
                    excess, keep = waits[:-1], waits[-1:]
                    for i in range(0, len(excess), 2):
                        ev = mybir.InstEventSemaphore(
                            name=f"EVW-{inst.name}-{i}", engine=inst.engine
                        )
                        ev.sync_info = mybir.SyncInfo(
                            on_wait=excess[i:i + 2], on_update=[]
                        )
                        new.append(ev)
                        n += 1
                    inst.sync_info = mybir.SyncInfo(
                        on_wait=keep,
                        on_update=list(si.on_update) if si.on_update else [],
                    )
                    changed = True
                new.append(inst)
            if changed:
                insts[:] = new
    return n


def build_nc(repeat=1, mode="full", chunks=CHUNKS):
    starts = np.cumsum([0] + list(chunks))[:-1]
    nc = bass.Bass("TRN2", target_bir_lowering=False, debug=False)

    # x planes, chunk-contiguous; h = kt*128 + p
    XHs = [
        nc.dram_tensor(f"XH{c}", [128, KT, cs], F16, kind="ExternalInput").ap()
        for c, cs in enumerate(chunks)
    ]
    XLs = [
        nc.dram_tensor(
            f"XL{c}", [128, KP, 2, cs], F8E4, kind="ExternalInput").ap()
        for c, cs in enumerate(chunks)
    ]
    WH = nc.dram_tensor("WH", [128, KT, E], F16, kind="ExternalInput").ap()
    W8A = nc.dram_tensor("W8A", [128, KP, 2, E], F8E4, kind="ExternalInput").ap()
    W8B = nc.dram_tensor("W8B", [128, KP, 2, E], F8E5, kind="ExternalInput").ap()
    B2 = nc.dram_tensor("B2", [128, E], F32, kind="ExternalInput").ap()

    OIDX = nc.dram_tensor("OIDX", [TPC, 8], U32, kind="ExternalOutput").ap()
    OWTS = nc.dram_tensor("OWTS", [TPC, 8], F32, kind="ExternalOutput").ap()

    with tile.TileContext(nc) as tc:
        with (
            tc.tile_pool(name="const", bufs=1) as cpool,
            tc.tile_pool(name="xh", bufs=2) as xhpool,
            tc.tile_pool(name="xl", bufs=2) as xlpool,
            tc.tile_pool(name="pm", bufs=2, space="PSUM") as pmpool,
            tc.tile_pool(name="pt", bufs=3, space="PSUM") as ptpool,
            tc.tile_pool(name="ts", bufs=2) as tpool,
            tc.tile_pool(name="sc", bufs=2) as spool,
            tc.tile_pool(name="sm", bufs=3) as smpool,
            tc.tile_pool(name="out", bufs=1) as opool,
        ):
            # weights DMA'd once, outside the repeat loop
            wh = cpool.tile([128, KT, E], F16, name="wh")
            nc.sync.dma_start(wh[:], WH)
            w8a = cpool.tile([128, KP, 2, E], F8E4, name="w8a")
            nc.sync.dma_start(w8a[:], W8A)
            w8b = cpool.tile([128, KP, 2, E], F8E5, name="w8b")
            nc.sync.dma_start(w8b[:], W8B)
            bias2 = cpool.tile([128, E], F32, name="bias2")
            nc.sync.dma_start(bias2[:], B2)
            ident = cpool.tile([128, 128], F32, name="ident")
            make_identity(nc, ident[:])

            oidx = opool.tile([128, TPC // 128, 8], U32, name="oidx")
            owts = opool.tile([128, TPC // 128, 8], F32, name="owts")

            xh_tiles = {}   # c -> tile [128, KT, cs]
            xl_tiles = {}   # c -> tile [128, KP, 2, cs]
            pms = {}        # c -> [pm0, pm1]

            def dma_xh(c):
                cs = chunks[c]
                t = xhpool.tile([128, KT, cs], F16, tag="xh", name=f"xh_{c}")
                xh_tiles[c] = t
                nc.sync.dma_start(t[:], XHs[c])

            def dma_xl(c):
                cs = chunks[c]
                t = xlpool.tile([128, KP, 2, cs], F8E4, tag="xl",
                                name=f"xl_{c}")
                xl_tiles[c] = t
                nc.sync.dma_start(t[:], XLs[c])

            def mm_phase(c):
                cs = chunks[c]
                pm = [
                    pmpool.tile([128, 512], F32, tag=f"pm{e}",
                                name=f"pm{e}_{c}")
                    for e in (0, 1)
                ]
                pms[c] = pm
                xh = xh_tiles.pop(c)
                xt = xl_tiles.pop(c)
                # fp16 pass; kt0 opens the full-width accumulation group
                for kt in range(KT):
                    for e in (0, 1):
                        nc.tensor.matmul(
                            pm[e][:, 0:cs],
                            wh[:, kt, e * 128:(e + 1) * 128],
                            xh[:, kt, :],
                            start=(kt == 0), stop=False,
                        )
                # DR-A, chunk-wide moving [2, cs]
                for kp in range(KP):
                    for e in (0, 1):
                        nc.tensor.matmul(
                            pm[e][:, 0:cs],
                            w8a[:, kp, :, e * 128:(e + 1) * 128],
                            xt[:, kp, :, :],
                            start=False, stop=False,
                            perf_mode=DR,
                        )
                # DR-B: e5m2 high-byte view of the fp16 plane; kp27 closes
                # the accumulation group
                xv = xh[:].bitcast(F8E5).rearrange("p k (n b) -> p k b n", b=2)
                for kp in range(KP):
                    for e in (0, 1):
                        nc.tensor.matmul(
                            pm[e][:, 0:cs],
                            w8b[:, kp, :, e * 128:(e + 1) * 128],
                            xv[:, 2 * kp:2 * kp + 2, 1, :],
                            start=False, stop=(kp == KP - 1),
                            perf_mode=DR,
                        )

            def post_phase(c):
                cs = chunks[c]
                if mode in ("mm_only",):
                    pms.pop(c)
                    return
                pm = pms.pop(c)
                # tanh(x.w/2) -> SBUF, [e, tok] layout
                ts = []
                for e in (0, 1):
                    t = tpool.tile([128, 512], F32, tag="ts", name=f"t{e}_{c}")
                    nc.scalar.activation(
                        t[:, 0:cs], pm[e][:, 0:cs],
                        mybir.ActivationFunctionType.Tanh,
                        scale=ACT_SCALE,
                    )
                    ts.append(t)
                # phase A: transpose to [tok, e] and add bias -- kept ahead
                # of the long DVE chains so PE never stalls on DVE slots
                s2s = []
                for j in range(cs // 128):
                    pt = ptpool.tile([128, E], F32, tag="pt",
                                     name=f"pt_{c}_{j}")
                    for e in (0, 1):
                        nc.tensor.matmul(
                            pt[:, e * 128:(e + 1) * 128],
                            ts[e][:, j * 128:(j + 1) * 128],
                            ident[:],
                            is_transpose=True,
                            start=(e == 0),
                            stop=(e == 1),
                        )
                    # s2 = 2*sigmoid + 2*bias = tanh + (1 + 2*bias)
                    s2 = spool.tile([128, E], F32, tag="s2",
                                    name=f"s2_{c}_{j}")
                    nc.vector.tensor_tensor(
                        s2[:], pt[:], bias2[:], op=mybir.AluOpType.add
                    )
                    s2s.append(s2)
                if mode == "half_post":
                    return
                # phase B: per-token hierarchical top-k (pure DVE; overlaps
                # the next chunk's matmuls)
                for j in range(cs // 128):
                    tok0 = int(starts[c]) // 128 + j
                    s2 = s2s[j]
                    # group scores: top-2 sum within each group of 32
                    g8 = smpool.tile([128, G, 8], F32, tag="g8",
                                     name=f"g8_{c}_{j}")
                    for g in range(G):
                        nc.vector.max(
                            out=g8[:, g, :], in_=s2[:, g * EPG:(g + 1) * EPG]
                        )
                    gs = smpool.tile([128, G], F32, tag="gs",
                                     name=f"gs_{c}_{j}")
                    nc.vector.reduce_sum(
                        gs[:], g8[:, :, 0:2], axis=mybir.AxisListType.X
                    )
                    gss = smpool.tile([128, G], F32, tag="gss",
                                      name=f"gss_{c}_{j}")
                    nc.vector.max(out=gss[:], in_=gs[:])
                    gmask = smpool.tile([128, G], F32, tag="gmask",
                                        name=f"gm_{c}_{j}")
                    nc.vector.tensor_scalar(
                        gmask[:], gs[:], gss[:, 3:4], None,
                        op0=mybir.AluOpType.is_ge,
                    )
                    s2m = spool.tile([128, E], F32, tag="s2m",
                                     name=f"s2m_{c}_{j}")
                    nc.vector.tensor_tensor(
                        s2m[:].rearrange("p (g e) -> p g e", g=G),
                        s2[:].rearrange("p (g e) -> p g e", g=G),
                        gmask[:].to_broadcast([128, G, EPG]),
                        op=mybir.AluOpType.mult,
                    )
                    # top-8 experts
                    mx = smpool.tile([128, 8], F32, tag="mx",
                                     name=f"mx_{c}_{j}")
                    nc.vector.max(out=mx[:], in_=s2m[:])
                    nc.vector.max_index(
                        out=oidx[:, tok0, :], in_max=mx[:], in_values=s2m[:]
                    )
                    # normalize: w = mx / sum(mx) * 2.5
                    sm = smpool.tile([128, 1], F32, tag="sm",
                                     name=f"sm_{c}_{j}")
                    nc.vector.reduce_sum(sm[:], mx[:], axis=mybir.AxisListType.X)
                    rc = smpool.tile([128, 1], F32, tag="rc",
                                     name=f"rc_{c}_{j}")
                    nc.vector.reciprocal(rc[:], sm[:])
                    nc.vector.tensor_scalar(
                        owts[:, tok0, :], mx[:], rc[:, 0:1], ROUTED_SCALING,
                        op0=mybir.AluOpType.mult, op1=mybir.AluOpType.mult,
                    )

            def trace_all():
                # xh double-buffered (prefetch c+1 during c); xl single-
                # buffered: chunk c+1's xl DMA is issued after chunk c's
                # DR-A (xl's last reader) and only needs to land by c+1's
                # DR-A, ~60% through its PE stream. post(c-1) is emitted
                # after mm(c) so PE transposes/DVE top-k have slack.
                dma_xh(0)
                dma_xl(0)
                dma_xh(1)
                for c in range(len(chunks)):
                    mm_phase(c)
                    if c + 1 < len(chunks):
                        dma_xl(c + 1)
                    if c + 2 < len(chunks):
                        dma_xh(c + 2)
                    if c > 0:
                        post_phase(c - 1)
                post_phase(len(chunks) - 1)

            if mode != "full":
                nc.gpsimd.memset(oidx[:], 0)
                nc.gpsimd.memset(owts[:], 0.0)
            if repeat == 1:
                trace_all()
            else:
                with tc.For_i(0, repeat, 1):
                    trace_all()

            nc.sync.dma_start(
                OIDX.rearrange("(t p) k -> p t k", p=128), oidx[:]
            )
            nc.sync.dma_start(
                OWTS.rearrange("(t p) k -> p t k", p=128), owts[:]
            )

    _split_caps(nc)
    return nc


def prep_inputs(hidden_states, weight, bias, chunks=CHUNKS):
    """Host-side: scale, fp16+fp8 plane split, transpose, per-core layout."""
    x = np.ascontiguousarray(hidden_states, dtype=np.float32).reshape(N_TOK, H)

    wf = weight.astype(np.float32)
    wh_f32 = (wf * SCALE_W).astype(np.float16).astype(np.float32)   # w*2^11 rounded
    # WH [H, E] fp16 -> [128, KT, E]
    WHm = np.ascontiguousarray(
        wh_f32.astype(np.float16).reshape(KT, 128, E).transpose(1, 0, 2))
    # W8A = e4m3(w*2^3) -> [128, KP, 2, E]
    W8Am = np.ascontiguousarray(
        (wf * 8.0).astype(NP_E4).reshape(KP, 2, 128, E).transpose(2, 0, 1, 3))
    # W8B = e5m2(w*2^11 - wh) -> [128, KP, 2, E]
    wlo = wf * SCALE_W - wh_f32
    W8Bm = np.ascontiguousarray(
        wlo.astype(NP_E5).reshape(KP, 2, 128, E).transpose(2, 0, 1, 3))

    b2 = (1.0 + 2.0 * bias.astype(np.float32))[None, :]
    b2 = np.ascontiguousarray(np.broadcast_to(b2, (128, E)))

    starts = np.cumsum([0] + list(chunks))[:-1]
    in_maps = []
    for c in range(N_CORES):
        xc = x[c * TPC:(c + 1) * TPC] * SCALE_X          # [TPC, H] f32
        xh = xc.astype(np.float16)
        r8 = ((xc - xh.astype(np.float32)) * 256.0).astype(NP_E4)
        xht = xh.T.reshape(KT, 128, TPC)                 # [KT, 128, TPC]
        r8t = r8.T.reshape(KP, 2, 128, TPC)              # [KP, 2, 128, TPC]
        im = dict(WH=WHm, W8A=W8Am, W8B=W8Bm, B2=b2)
        for ci, cs in enumerate(chunks):
            t0 = int(starts[ci])
            im[f"XH{ci}"] = np.ascontiguousarray(
                xht[:, :, t0:t0 + cs].transpose(1, 0, 2))
            im[f"XL{ci}"] = np.ascontiguousarray(
                r8t[:, :, :, t0:t0 + cs].transpose(2, 0, 1, 3))
        in_maps.append(im)
    return in_maps


_NC_CACHE = {}


def kernel(hidden_states, weight, bias):
    key = "main"
    if key not in _NC_CACHE:
        _NC_CACHE[key] = build_nc()
    nc = _NC_CACHE[key]
    in_maps = prep_inputs(hidden_states, weight, bias)
    res = run_bass_kernel_spmd(nc, in_maps, core_ids=list(range(N_CORES)))
    idx = np.concatenate(
        [r["OIDX"].astype(np.int32) for r in res.results], axis=0
    ).reshape(N_TOK, 8)
    wts = np.concatenate([r["OWTS"] for r in res.results], axis=0).reshape(N_TOK, 8)
    return idx, wts
